# revision 1
# baseline (speedup 1.0000x reference)
"""CharRNN Trainium2 kernel (8-core data-parallel).

Math: h_t = tanh(emb[x_t] @ Wx + h_{t-1} @ Wh + b_rnn); logits = (h_T * mask) @ Wd + bd.

Key transformation: emb[x] @ Wx == (emb @ Wx)[x], so the embedding (V=256, E=50)
and input projection fold into one tiny table M = emb @ Wx + b_rnn of shape
[256, 10]. The host gathers U = M[x] per batch shard (indexing only — all FLOPs
beyond the 256x50x10 table build run on device... the table build itself is
128K MACs, negligible) and ships U in the exact on-chip layout the scan wants.

Device layout (per core, batch shard 2048 padded to 2052 = 12 groups x 171):
  partitions 10g+h (g in [0,12), h in [0,10)) hold hidden unit h of batch
  group g; the free dim holds the 171 batch lanes of that group. The RNN
  matmul uses a block-diagonal Wh [120,120], so one PE matmul + one ACT tanh
  advances all 2048 batch elements one time step. U enters PSUM via an
  identity matmul (one N=342 deposit per 2-step bank); the Wh matmul then
  accumulates on top (start=False). NOTE: a DVE tensor_copy into PSUM with a
  PE accumulate on top races nondeterministically on HW — only PE writes PSUM.
"""
import numpy as np

import concourse.bass as bass
import concourse.mybir as mybir
from concourse.tile import TileContext
from concourse.bass_utils import run_bass_kernel_spmd

# problem shape (hardcoded per contract)
B, T, V, E, H, L = 16384, 100, 256, 50, 10, 15
N_CORES = 8
BC = B // N_CORES          # 2048 batch per core
G = 12                     # partition groups
BG = 171                   # batch lanes per group
BP = G * BG                # 2052 padded batch per core
NF = T * BG                # u free dim = 17100
# u is DMA'd in T-chunks overlapping the scan; first chunks are small so
# the scan starts almost immediately
CHUNK_STEPS = [2, 2, 4, 8, 12, 16, 16, 20, 20]
assert sum(CHUNK_STEPS) == T

F32 = mybir.dt.float32


def _split_multi_waits(nc):
    """This walrus build rejects >1 sem wait per instruction; hoist extras
    onto NoOps just before, on the same (in-order) engine queue."""
    uid = 0
    for f in nc.m.functions:
        for bb in f.blocks:
            if not any(
                i.sync_info is not None and len(i.sync_info.on_wait) > 1
                for i in bb.instructions
            ):
                continue
            new_list = []
            for inst in bb.instructions:
                si = inst.sync_info
                if si is not None and len(si.on_wait) > 1:
                    waits = list(si.on_wait)
                    for w in waits[:-1]:
                        uid += 1
                        new_list.append(
                            mybir.InstNoOp(
                                name=f"WS-{uid}",
                                engine=inst.engine,
                                bass_nofuse=True,
                                sync_info=mybir.SyncInfo(on_wait=[w], on_update=[]),
                            )
                        )
                    inst.sync_info = mybir.SyncInfo(
                        on_wait=[waits[-1]], on_update=list(si.on_update)
                    )
                new_list.append(inst)
            bb.instructions = new_list


_NC_CACHE = None


def _build_nc():
    global _NC_CACHE
    if _NC_CACHE is not None:
        return _NC_CACHE
    nc = bass.Bass(trn_type="TRN2")
    u_d = nc.dram_tensor("u", [G * H, NF], F32, kind="ExternalInput")
    eye_d = nc.dram_tensor("eye", [G * H, G * H], F32, kind="ExternalInput")
    wh_d = nc.dram_tensor("wh", [G * H, G * H], F32, kind="ExternalInput")
    wd_d = nc.dram_tensor("wd", [G * H, 180], F32, kind="ExternalInput")
    bdv_d = nc.dram_tensor("bdv", [90, 1], F32, kind="ExternalInput")
    mask_d = nc.dram_tensor("mask", [G * H, BG], F32, kind="ExternalInput")
    o_d = [
        nc.dram_tensor(f"o{i}", [90, BG], F32, kind="ExternalOutput") for i in range(2)
    ]

    with TileContext(nc) as tc:
        with (
            tc.tile_pool(name="const", bufs=1) as cpool,
            tc.tile_pool(name="u", bufs=1) as upool,
            tc.tile_pool(name="work", bufs=4) as wpool,
            tc.tile_pool(name="psum", bufs=6, space="PSUM") as ppool,
            tc.tile_pool(name="psum2", bufs=2, space="PSUM") as ppool2,
        ):
            t_wh = cpool.tile([G * H, G * H], F32, tag="wh")
            nc.sync.dma_start(out=t_wh[:], in_=wh_d[:])
            t_eye = cpool.tile([G * H, G * H], F32, tag="eye")
            nc.sync.dma_start(out=t_eye[:], in_=eye_d[:])
            t_wd = cpool.tile([G * H, 180], F32, tag="wd")
            nc.sync.dma_start(out=t_wd[:], in_=wd_d[:])
            t_bdv = cpool.tile([90, 1], F32, tag="bdv")
            nc.sync.dma_start(out=t_bdv[:], in_=bdv_d[:])
            t_mask = cpool.tile([G * H, BG], F32, tag="mask")
            nc.sync.dma_start(out=t_mask[:], in_=mask_d[:])

            # warm the ACT tanh table while the first u chunk streams in
            warm = cpool.tile([128, 4], F32, tag="warm")
            nc.vector.memset(warm[:], 0.0)
            nc.scalar.activation(
                warm[:], warm[:], mybir.ActivationFunctionType.Tanh
            )

            # u chunk tiles: step t lives in chunk k at local step offset
            uts = []          # (tile, start_step) per chunk
            step0 = 0
            for k, ns in enumerate(CHUNK_STEPS):
                ut = upool.tile([G * H, ns * BG], F32, tag=f"u{k}")
                nc.sync.dma_start(
                    out=ut[:], in_=u_d[:, step0 * BG:(step0 + ns) * BG]
                )
                uts.append((ut, step0))
                step0 += ns
            step_src = {}
            for (ut, s0), ns in zip(uts, CHUNK_STEPS):
                for s in range(ns):
                    step_src[s0 + s] = (ut, s)

            h_cur = wpool.tile([G * H, BG], F32, tag="h")
            nc.vector.memset(h_cur[:], 0.0)

            # 2 time steps per PSUM bank: one eye-matmul (N=342) deposits
            # u for both steps, then per step one Wh accumulate + tanh.
            for p in range(T // 2):
                ps = ppool.tile([G * H, 2 * BG], F32, tag="ps")
                ut0, s0 = step_src[2 * p]
                ut1, s1 = step_src[2 * p + 1]
                if ut0 is ut1:
                    nc.tensor.matmul(
                        ps[:], t_eye[:], ut0[:, s0 * BG:(s0 + 2) * BG],
                        start=True, stop=False,
                    )
                else:
                    nc.tensor.matmul(
                        ps[:, 0:BG], t_eye[:], ut0[:, s0 * BG:(s0 + 1) * BG],
                        start=True, stop=False,
                    )
                    nc.tensor.matmul(
                        ps[:, BG:2 * BG], t_eye[:], ut1[:, s1 * BG:(s1 + 1) * BG],
                        start=True, stop=False, skip_group_check=True,
                    )
                for s in range(2):
                    sl = ps[:, s * BG:(s + 1) * BG]
                    nc.tensor.matmul(
                        sl, t_wh[:], h_cur[:],
                        start=False, stop=True, skip_group_check=True,
                    )
                    h_new = wpool.tile([G * H, BG], F32, tag="h")
                    nc.scalar.activation(
                        h_new[:], sl, mybir.ActivationFunctionType.Tanh
                    )
                    h_cur = h_new

            hm = wpool.tile([G * H, BG], F32, tag="hm")
            nc.vector.tensor_mul(hm[:], h_cur[:], t_mask[:])
            for half in range(2):
                po = ppool2.tile([90, BG], F32, tag="po")
                nc.tensor.matmul(
                    po[:], t_wd[:, 90 * half:90 * (half + 1)], hm[:],
                    start=True, stop=True,
                )
                ob = wpool.tile([90, BG], F32, tag=f"ob{half}")
                nc.vector.tensor_scalar_add(ob[:], po[:], t_bdv[:])
                nc.sync.dma_start(out=o_d[half][:], in_=ob[:])

    _split_multi_waits(nc)
    _NC_CACHE = nc
    return nc


def _prepare_in_maps(x, emb, Wx, Wh, b_rnn, Wd, bd, drop_mask):
    x = np.asarray(x)
    emb = np.asarray(emb, dtype=np.float32)
    Wx = np.asarray(Wx, dtype=np.float32)
    Wh = np.asarray(Wh, dtype=np.float32)
    b_rnn = np.asarray(b_rnn, dtype=np.float32)
    Wd = np.asarray(Wd, dtype=np.float32)
    bd = np.asarray(bd, dtype=np.float32)
    drop_mask = np.asarray(drop_mask, dtype=np.float32)

    M = emb @ Wx + b_rnn  # [V, H] fused embedding+input-proj table

    wh_blk = np.zeros((G * H, G * H), np.float32)
    wd_blk = np.zeros((G * H, 180), np.float32)
    for a in range(G):
        wh_blk[10 * a:10 * a + 10, 10 * a:10 * a + 10] = Wh
        half, b6 = divmod(a, 6)
        wd_blk[10 * a:10 * a + 10, 90 * half + 15 * b6:90 * half + 15 * b6 + 15] = Wd
    bdv = np.tile(bd, 6).reshape(90, 1).astype(np.float32)

    in_maps = []
    for c in range(N_CORES):
        xs = x[c * BC:(c + 1) * BC].astype(np.int64)
        u = np.zeros((BP, T, H), np.float32)
        u[:BC] = M[xs]
        # [120, 17100]: u_dev[10g+h, 171t+j] = u[171g+j, t, h]
        u_dev = np.ascontiguousarray(
            u.reshape(G, BG, T, H).transpose(0, 3, 2, 1).reshape(G * H, NF)
        )
        mp = np.zeros((BP, H), np.float32)
        mp[:BC] = drop_mask[c * BC:(c + 1) * BC]
        mask_dev = np.ascontiguousarray(
            mp.reshape(G, BG, H).transpose(0, 2, 1).reshape(G * H, BG)
        )
        in_maps.append(
            {"u": u_dev, "eye": np.eye(G * H, dtype=np.float32), "wh": wh_blk,
             "wd": wd_blk, "bdv": bdv, "mask": mask_dev}
        )
    return in_maps


def _assemble(results):
    logits = np.empty((B, L), np.float32)
    for c in range(N_CORES):
        parts = []
        for half in range(2):
            o = results[c][f"o{half}"]  # [90, 171]
            parts.append(o.reshape(6, 15, BG).transpose(0, 2, 1).reshape(6 * BG, 15))
        full = np.concatenate(parts, axis=0)  # [2052, 15]
        logits[c * BC:(c + 1) * BC] = full[:BC]
    return logits


_LAST_RES = None


def kernel(x, emb, Wx, Wh, b_rnn, Wd, bd, drop_mask, _trace=False):
    global _LAST_RES
    nc = _build_nc()
    in_maps = _prepare_in_maps(x, emb, Wx, Wh, b_rnn, Wd, bd, drop_mask)
    res = run_bass_kernel_spmd(
        nc, in_maps, core_ids=list(range(N_CORES)), trace=_trace
    )
    _LAST_RES = res
    out = _assemble(res.results)
    if _trace:
        kernel.last_exec_time_ns = res.exec_time_ns
    return out



# revision 2
# speedup vs baseline: 1.2694x; 1.2694x over previous
"""CharRNN Trainium2 kernel (8-core data-parallel), bf16 scan.

Math: h_t = tanh(emb[x_t] @ Wx + h_{t-1} @ Wh + b_rnn); logits = (h_T * mask) @ Wd + bd.

Key transformation: emb[x] @ Wx == (emb @ Wx)[x], so the embedding (V=256, E=50)
and input projection fold into one tiny table M = emb @ Wx + b_rnn of shape
[256, 10]. The host gathers U = M[x] per batch shard (indexing only) and ships
U in the exact on-chip layout the scan wants, as bf16 (halves DMA and enables
1-cycle/row PE matmuls vs fp32's 4).

Device layout (per core, batch shard 2048 padded to 2052 = 12 groups x 171):
  partitions 10g+h (g in [0,12), h in [0,10)) hold hidden unit h of batch
  group g; the free dim holds the 171 batch lanes of that group. The RNN
  matmul uses a block-diagonal Wh [120,120] bf16, so one PE matmul + one ACT
  tanh advances all 2048 batch elements one time step. U enters PSUM via an
  identity matmul (one N=342 bf16 deposit per 2-step bank); the Wh matmul then
  accumulates on top (start=False). h is carried in bf16 (validated on host:
  final logit rel err ~8e-3 vs the 2e-2 gate); the last step's tanh and the
  dense head stay f32. NOTE: only PE may write PSUM (DVE copy races on HW).
"""
import numpy as np
import ml_dtypes

import concourse.bass as bass
import concourse.mybir as mybir
from concourse.tile import TileContext
from concourse.bass_utils import run_bass_kernel_spmd

# problem shape (hardcoded per contract)
B, T, V, E, H, L = 16384, 100, 256, 50, 10, 15
N_CORES = 8
BC = B // N_CORES          # 2048 batch per core
G = 12                     # partition groups
BG = 171                   # batch lanes per group
BP = G * BG                # 2052 padded batch per core
NF = T * BG                # u free dim = 17100
# u is DMA'd in T-chunks overlapping the scan; first chunks are small so
# the scan starts almost immediately
CHUNK_STEPS = [2, 2, 4, 8, 12, 16, 16, 20, 20]
assert sum(CHUNK_STEPS) == T

F32 = mybir.dt.float32
BF16 = mybir.dt.bfloat16
NP_BF16 = ml_dtypes.bfloat16


def _split_multi_waits(nc):
    """This walrus build rejects >1 sem wait per instruction; hoist extras
    onto NoOps just before, on the same (in-order) engine queue."""
    uid = 0
    for f in nc.m.functions:
        for bb in f.blocks:
            if not any(
                i.sync_info is not None and len(i.sync_info.on_wait) > 1
                for i in bb.instructions
            ):
                continue
            new_list = []
            for inst in bb.instructions:
                si = inst.sync_info
                if si is not None and len(si.on_wait) > 1:
                    waits = list(si.on_wait)
                    for w in waits[:-1]:
                        uid += 1
                        new_list.append(
                            mybir.InstNoOp(
                                name=f"WS-{uid}",
                                engine=inst.engine,
                                bass_nofuse=True,
                                sync_info=mybir.SyncInfo(on_wait=[w], on_update=[]),
                            )
                        )
                    inst.sync_info = mybir.SyncInfo(
                        on_wait=[waits[-1]], on_update=list(si.on_update)
                    )
                new_list.append(inst)
            bb.instructions = new_list


_NC_CACHE = None


def _build_nc():
    global _NC_CACHE
    if _NC_CACHE is not None:
        return _NC_CACHE
    nc = bass.Bass(trn_type="TRN2")
    u_d = nc.dram_tensor("u", [G * H, NF], BF16, kind="ExternalInput")
    eye_d = nc.dram_tensor("eye", [G * H, G * H], BF16, kind="ExternalInput")
    wh_d = nc.dram_tensor("wh", [G * H, G * H], BF16, kind="ExternalInput")
    wd_d = nc.dram_tensor("wd", [G * H, 180], F32, kind="ExternalInput")
    bdv_d = nc.dram_tensor("bdv", [90, 1], F32, kind="ExternalInput")
    mask_d = nc.dram_tensor("mask", [G * H, BG], F32, kind="ExternalInput")
    o_d = [
        nc.dram_tensor(f"o{i}", [90, BG], F32, kind="ExternalOutput") for i in range(2)
    ]

    with TileContext(nc) as tc:
        with (
            tc.tile_pool(name="const", bufs=1) as cpool,
            tc.tile_pool(name="u", bufs=1) as upool,
            tc.tile_pool(name="work", bufs=4) as wpool,
            tc.tile_pool(name="psum", bufs=6, space="PSUM") as ppool,
            tc.tile_pool(name="psum2", bufs=2, space="PSUM") as ppool2,
        ):
            t_wh = cpool.tile([G * H, G * H], BF16, tag="wh")
            nc.sync.dma_start(out=t_wh[:], in_=wh_d[:])
            t_eye = cpool.tile([G * H, G * H], BF16, tag="eye")
            nc.sync.dma_start(out=t_eye[:], in_=eye_d[:])
            t_wd = cpool.tile([G * H, 180], F32, tag="wd")
            nc.sync.dma_start(out=t_wd[:], in_=wd_d[:])
            t_bdv = cpool.tile([90, 1], F32, tag="bdv")
            nc.sync.dma_start(out=t_bdv[:], in_=bdv_d[:])
            t_mask = cpool.tile([G * H, BG], F32, tag="mask")
            nc.sync.dma_start(out=t_mask[:], in_=mask_d[:])

            # warm the ACT tanh table while the first u chunk streams in
            warm = cpool.tile([128, 4], F32, tag="warm")
            nc.vector.memset(warm[:], 0.0)
            nc.scalar.activation(
                warm[:], warm[:], mybir.ActivationFunctionType.Tanh
            )

            # u chunk tiles: step t lives in chunk k at local step offset
            uts = []          # (tile, start_step) per chunk
            step0 = 0
            for k, ns in enumerate(CHUNK_STEPS):
                ut = upool.tile([G * H, ns * BG], BF16, tag=f"u{k}")
                nc.sync.dma_start(
                    out=ut[:], in_=u_d[:, step0 * BG:(step0 + ns) * BG]
                )
                uts.append((ut, step0))
                step0 += ns
            step_src = {}
            for (ut, s0), ns in zip(uts, CHUNK_STEPS):
                for s in range(ns):
                    step_src[s0 + s] = (ut, s)

            h_cur = wpool.tile([G * H, BG], BF16, tag="h")
            nc.vector.memset(h_cur[:], 0.0)

            # 2 time steps per PSUM bank: one eye-matmul (N=342) deposits
            # u for both steps, then per step one Wh accumulate + tanh.
            for p in range(T // 2):
                ps = ppool.tile([G * H, 2 * BG], F32, tag="ps")
                ut0, s0 = step_src[2 * p]
                ut1, s1 = step_src[2 * p + 1]
                if ut0 is ut1:
                    nc.tensor.matmul(
                        ps[:], t_eye[:], ut0[:, s0 * BG:(s0 + 2) * BG],
                        start=True, stop=False,
                    )
                else:
                    nc.tensor.matmul(
                        ps[:, 0:BG], t_eye[:], ut0[:, s0 * BG:(s0 + 1) * BG],
                        start=True, stop=False,
                    )
                    nc.tensor.matmul(
                        ps[:, BG:2 * BG], t_eye[:], ut1[:, s1 * BG:(s1 + 1) * BG],
                        start=True, stop=False, skip_group_check=True,
                    )
                for s in range(2):
                    step = 2 * p + s
                    sl = ps[:, s * BG:(s + 1) * BG]
                    nc.tensor.matmul(
                        sl, t_wh[:], h_cur[:],
                        start=False, stop=True, skip_group_check=True,
                    )
                    last = step == T - 1
                    h_new = wpool.tile(
                        [G * H, BG], F32 if last else BF16, tag="hf" if last else "h"
                    )
                    nc.scalar.activation(
                        h_new[:], sl, mybir.ActivationFunctionType.Tanh
                    )
                    h_cur = h_new

            hm = wpool.tile([G * H, BG], F32, tag="hm")
            nc.vector.tensor_mul(hm[:], h_cur[:], t_mask[:])
            for half in range(2):
                po = ppool2.tile([90, BG], F32, tag="po")
                nc.tensor.matmul(
                    po[:], t_wd[:, 90 * half:90 * (half + 1)], hm[:],
                    start=True, stop=True,
                )
                ob = wpool.tile([90, BG], F32, tag=f"ob{half}")
                nc.vector.tensor_scalar_add(ob[:], po[:], t_bdv[:])
                nc.sync.dma_start(out=o_d[half][:], in_=ob[:])

    _split_multi_waits(nc)
    _NC_CACHE = nc
    return nc


def _prepare_in_maps(x, emb, Wx, Wh, b_rnn, Wd, bd, drop_mask):
    x = np.asarray(x)
    emb = np.asarray(emb, dtype=np.float32)
    Wx = np.asarray(Wx, dtype=np.float32)
    Wh = np.asarray(Wh, dtype=np.float32)
    b_rnn = np.asarray(b_rnn, dtype=np.float32)
    Wd = np.asarray(Wd, dtype=np.float32)
    bd = np.asarray(bd, dtype=np.float32)
    drop_mask = np.asarray(drop_mask, dtype=np.float32)

    M = emb @ Wx + b_rnn  # [V, H] fused embedding+input-proj table
    Mb = M.astype(NP_BF16)

    wh_blk = np.zeros((G * H, G * H), np.float32)
    wd_blk = np.zeros((G * H, 180), np.float32)
    for a in range(G):
        wh_blk[10 * a:10 * a + 10, 10 * a:10 * a + 10] = Wh
        half, b6 = divmod(a, 6)
        wd_blk[10 * a:10 * a + 10, 90 * half + 15 * b6:90 * half + 15 * b6 + 15] = Wd
    bdv = np.tile(bd, 6).reshape(90, 1).astype(np.float32)
    wh_blk = wh_blk.astype(NP_BF16)
    eye_blk = np.eye(G * H, dtype=NP_BF16)

    in_maps = []
    for c in range(N_CORES):
        xs = x[c * BC:(c + 1) * BC].astype(np.int64)
        u = np.zeros((BP, T, H), NP_BF16)
        u[:BC] = Mb[xs]
        # [120, 17100]: u_dev[10g+h, 171t+j] = u[171g+j, t, h]
        u_dev = np.ascontiguousarray(
            u.reshape(G, BG, T, H).transpose(0, 3, 2, 1).reshape(G * H, NF)
        )
        mp = np.zeros((BP, H), np.float32)
        mp[:BC] = drop_mask[c * BC:(c + 1) * BC]
        mask_dev = np.ascontiguousarray(
            mp.reshape(G, BG, H).transpose(0, 2, 1).reshape(G * H, BG)
        )
        in_maps.append(
            {"u": u_dev, "eye": eye_blk, "wh": wh_blk,
             "wd": wd_blk, "bdv": bdv, "mask": mask_dev}
        )
    return in_maps


def _assemble(results):
    logits = np.empty((B, L), np.float32)
    for c in range(N_CORES):
        parts = []
        for half in range(2):
            o = results[c][f"o{half}"]  # [90, 171]
            parts.append(o.reshape(6, 15, BG).transpose(0, 2, 1).reshape(6 * BG, 15))
        full = np.concatenate(parts, axis=0)  # [2052, 15]
        logits[c * BC:(c + 1) * BC] = full[:BC]
    return logits


_LAST_RES = None


def kernel(x, emb, Wx, Wh, b_rnn, Wd, bd, drop_mask, _trace=False):
    global _LAST_RES
    nc = _build_nc()
    in_maps = _prepare_in_maps(x, emb, Wx, Wh, b_rnn, Wd, bd, drop_mask)
    res = run_bass_kernel_spmd(
        nc, in_maps, core_ids=list(range(N_CORES)), trace=_trace
    )
    _LAST_RES = res
    out = _assemble(res.results)
    if _trace:
        kernel.last_exec_time_ns = res.exec_time_ns
    return out
